# revision 72
# baseline (speedup 1.0000x reference)
"""GraphVAE MPM kernel for Trainium2 (Bass/Tile), self-contained.

Math: the reference's S[i,j,a,b] tensor is separable off the overrides:
S = c_ij * Q[a,b] with c in {0,1}, so the per-iteration O(N^4) masked
max-product collapses to an O(N^3) grouped max (T1[j,a] = max_b Qz[a,b]*X[j,b],
clamped by G[j] = -1e6*min_{b>=R} X[j,b]) plus a 64x64 matmul with Cz.
Edge terms outside the real-node block reduce to per-row scalars built from
G via masked partition-sums (done as PE matmuls against static 0/1 matrices).

Key structural optimizations:
- The whole iteration map is positively 1-homogeneous in X, so the exact L2
  normalization can be replaced in-loop by ANY positive per-iteration scale
  (one exact normalize after the loop reproduces the reference bit-for-bit
  up to fp noise).  The scale used is 1/|sum_j G[j]| -- a magnitude proxy
  reusing the already-needed Sg partition-sum.  Its reciprocal is issued
  right AFTER the u product in the DVE stream (its input |sum g| is ready
  ~850ns after the previous tail update, i.e. before u finishes), so it
  never gates the next body's u the way an end-of-stream recip does.
- The per-row additive correction (psc0 + tsel - psc2 in the original
  formulation) is LINEAR in t0=relu(G) and G, so it folds into TWO
  accumulating PE matmuls against precomputed constants
  (selin - Cz - diag_in  and  selout - diag_out).  On hardware every ACT op
  costs ~190ns (224-cycle SBUF bubble) and each cross-engine hop ~100ns, so
  the original 9-hop scalar chain (g->t0->psc->e_sel->esel2->esel3->xna2)
  sat ON the critical cycle and dominated the iteration; the fused form is
  4 hops with ~1.5us of slack against the big DVE chain.
- t0 runs on GPSIMD in parallel with g on ACT (both direct from tmin2).
- The N-R tail columns of X are identical every iteration (equal init and
  an identical per-column affine recurrence), so the tail state IS the
  single scalar tmin2, updated as invn*(-1e6*tmin2 + e_tail) on ACT+GPSIMD;
  the [128,8] DVE tail update and its min-accumulator disappear, and the
  full tail block is reconstructed by broadcast only for the final output.
- The first Cz matmul contracts K=64 (its weight block is zero on
  partitions 64:128), halving that Ldweights stream on the critical
  reduce->matmul->x-update path.
- The big O(N^3) product+max runs on ALL 128 DVE partitions: partition
  p = j + 64*h covers a-half h.  The product runs in bf16 (2x DVE mode);
  a bf16 tensor_tensor max pre-folds the b-axis in half (2x) before the
  1x tensor_reduce.  The G clamp rides in a trailing column of the folded
  tensor so the reduce applies it for free.  (Chunking these ops below the
  DVE drain threshold was measured SLOWER on this part -- per-instruction
  overhead ~125ns dominates -- so the ops stay whole.)
- The state x is CARRIED IN BF16 (it only feeds the bf16 product, the
  gpsimd node-term mult, and the final normalize), so the old x->x2 bf16
  copy disappears from the DVE stream.
- Small ops are spread over ACT/PE/GPSIMD; no ACT function outside the
  exp_and_others table set is used inside the loop (a Sqrt would force a
  ~2.7us ACT table reload per iteration; the one true Sqrt runs once after
  the loop).
- The 20 iterations run as a For_i hardware loop, fully unrolled inside one
  loop body per back-edge.

The same program is replicated SPMD on all 8 cores (the problem is a single
small graph; a cross-core split would need a ~12KB collective per iteration,
far slower than the iteration itself) and core 0's output is returned.
"""

import numpy as np

N = 64
R = 56
H = R // 2          # 28: a-half per partition group
HB = R // 2         # 28: b-fold width
W = R + 1           # 57: qz2 storage stride per a-group
ITERS = 20
UNROLL = 20
BIGNEG = -3.0e38
USE_BF16 = True

# --- tuning flags ---
XBF16 = True        # carry x in bf16 (drops the x->x2 copy op)
POOL_PSUM = False   # GPSIMD cannot access PSUM (BIR verifier); ACT copies first
POOL_T0 = True      # t0=relu(-1e6*tmin2) on gpsimd (parallel with g on ACT)
POOL_XT2 = False    # tail update on gpsimd instead of DVE
SCAN_MODE = False   # grouped max via one tensor_tensor_scan (mask-reset trick)
M_K64H = True       # first m matmul contracts K=64 (its czd block is zero on
                    # partitions 64:128), halving that Ldweights stream; the
                    # second block would need partition-offset operands, which
                    # the PE rejects at runtime, so it stays K=128
RECIP_AFTER_U = False  # recip parked in the post-reduce PE-wait gap: with the
                       # scalar tail the gap has no other filler, so the recip
                       # is free there instead of delaying v after u (paired
                       # A/B median -109ns/iter vs after-u)
SCALAR_TAIL = True  # all 8 tail columns of X are identical every iteration
                    # (equal init + identical affine recurrence), so the tail
                    # is ONE [128,1] scalar: tmin2 <- invn*(-1e6*tmin2 + e_tail)
                    # computed on ACT+GPSIMD; the DVE tail update disappears

_CACHE = {}


def _precompute(A_gt, vec_logits):
    """Host-side O(N^2) constant construction (mirrors reference's setup)."""
    import ml_dtypes

    bf16 = ml_dtypes.bfloat16 if USE_BF16 else np.float32
    A_gt = np.asarray(A_gt, np.float32)
    vec = np.asarray(vec_logits, np.float32)
    d = np.arange(N)

    iu = np.triu_indices(N, k=1)
    logits = np.zeros((N, N), np.float32)
    logits[iu] = vec
    logits = logits + logits.T
    logits[d, d] = np.float32(-10.0)
    B = (1.0 / (1.0 + np.exp(-logits))).astype(np.float32)

    A = A_gt.copy()
    r = int((A.sum(1) > 0).sum())
    real = d < r
    A[d, d] = np.where(real, np.float32(1.0), A[d, d])
    Bm = B.copy()
    Bm[d, d] = np.where(real, np.float32(1.0), Bm[d, d])
    dA = np.diagonal(A).copy()
    dB = np.diagonal(Bm).copy()
    degA = A.sum(1)
    degB = Bm.sum(1)
    node_sim = (1.0 / (np.abs(degA[:, None] - degB[None, :]) + 1.0)).astype(np.float32)

    Qz = (Bm * dB[:, None] * dB[None, :]).astype(np.float32)
    np.fill_diagonal(Qz, 0.0)

    # [128, H, W]: partition p=(h*64+j) holds Qz[28h+a', b] (j-independent),
    # with a zero in the trailing pad column.
    qz2 = np.zeros((128, H, W), np.float32)
    for h in range(2):
        qz2[64 * h:64 * (h + 1), :, :R] = Qz[28 * h:28 * (h + 1), :R][None, :, :]

    Cz = (A * dA[:, None] * dA[None, :]).astype(np.float32)
    np.fill_diagonal(Cz, 0.0)
    Cz[:, R:] = 0.0
    Cz[R:, :] = 0.0
    # [128, 256]: two K=128 weight blocks, each horizontally duplicated so
    # the m matmuls write the duplicated [128, 28] halves directly.
    # cols 0:128 = [Cz;0] tiled twice (a-half 0), cols 128:256 = [0;Cz] x2.
    czd = np.zeros((128, 4 * N), np.float32)
    czd[0:N, 0:N] = Cz
    czd[0:N, N:2 * N] = Cz
    czd[N:128, 2 * N:3 * N] = Cz
    czd[N:128, 3 * N:4 * N] = Cz

    ns = (dA[:, None] * dB[None, :] * node_sim).astype(np.float32)
    mask2 = (d[:, None] < R) & (d[None, :] < R)
    nsm = np.where(mask2, ns, np.float32(-1e6)).astype(np.float32)
    nsm128 = np.tile(nsm[:, :R], (2, 1))    # [128, R]

    # Fused per-row-correction weights.  Original chain:
    #   psc0 = selin@t0 + selout@g ; psc2 = cz@t0 ;
    #   tsel = -t0[p] (p<R) / -g[p] (p>=R) ;
    #   correction = psc0 + tsel - psc2
    # => correction = (selin - cz - diag_in)@t0 + (selout - diag_out)@g
    czh = np.tile(Cz, (1, 2))               # [64, 128]
    selin = np.zeros((N, 128), np.float32)
    selin[:, 0:R] = 1.0
    selin[:, N:N + R] = 1.0
    selout = np.zeros((N, 128), np.float32)
    selout[:, R:N] = 1.0
    selout[:, N + R:128] = 1.0
    diag_in = np.zeros((N, 128), np.float32)
    diag_out = np.zeros((N, 128), np.float32)
    for i in range(N):
        if i < R:
            diag_in[i, i] = 1.0
            diag_in[i, i + 64] = 1.0
        else:
            diag_out[i, i] = 1.0
            diag_out[i, i + 64] = 1.0
    selin2 = (selin - czh - diag_in).astype(np.float32)
    selout2 = (selout - diag_out).astype(np.float32)
    sel2 = np.concatenate([selin2, selout2], axis=1)   # [64, 256]

    qz264 = np.zeros((128, H, 64), np.float32)
    qz264[:, :, :W] = qz2

    return {
        "qz2p": qz2.reshape(128, H * W).astype(bf16),
        "qz2p64": qz264.reshape(128, H * 64).astype(bf16),
        "czdp": czd.astype(bf16),
        "czdp8": czd.astype(ml_dtypes.float8_e4m3),
        "sel2p": sel2,
        "nsmp": nsm128.astype(np.float32),
    }


def _build(iters=ITERS, xbf16=None, pool_psum=None, pool_t0=None,
           pool_xt2=None, scan_mode=None, diag_no_m=False, diag_no_xna2=False,
           act_tail=False, pe_warm=0, m_k64=False, m_k64h=None,
           czd_fp8=False, recip_after_u=None, scalar_tail=None, unroll=None,
           fold2=False, use_pool=False, red_split=False, aligned=False):
    import concourse.bass as bass
    import concourse.mybir as mybir
    from concourse import bacc
    from concourse.tile import TileContext

    if xbf16 is None:
        xbf16 = XBF16
    if pool_psum is None:
        pool_psum = POOL_PSUM
    if pool_t0 is None:
        pool_t0 = POOL_T0
    if pool_xt2 is None:
        pool_xt2 = POOL_XT2
    if scan_mode is None:
        scan_mode = SCAN_MODE
    if m_k64h is None:
        m_k64h = M_K64H
    if recip_after_u is None:
        recip_after_u = RECIP_AFTER_U
    if scalar_tail is None:
        scalar_tail = SCALAR_TAIL
    if unroll is None:
        unroll = UNROLL

    assert iters % unroll == 0
    f32 = mybir.dt.float32
    bf = mybir.dt.bfloat16 if USE_BF16 else mybir.dt.float32
    ALU = mybir.AluOpType
    ACTF = mybir.ActivationFunctionType
    AX = mybir.AxisListType

    xdt = bf if xbf16 else f32

    nc = bacc.Bacc()
    czdt = mybir.dt.float8e4 if czd_fp8 else bf
    WQ = 64 if aligned else W
    qz2p = nc.declare_dram_parameter("qz2p64" if aligned else "qz2p",
                                     [128, H * WQ], bf, isOutput=False)
    czdp = nc.declare_dram_parameter("czdp8" if czd_fp8 else "czdp",
                                     [128, 4 * N], czdt, isOutput=False)
    sel2p = nc.declare_dram_parameter("sel2p", [N, 256], f32, isOutput=False)
    nsmp = nc.declare_dram_parameter("nsmp", [128, R], f32, isOutput=False)
    xoutp = nc.declare_dram_parameter("xout", [N, N], f32, isOutput=True)

    with TileContext(nc) as tc:
        with (
            tc.tile_pool(name="consts", bufs=1) as cp,
            tc.tile_pool(name="big", bufs=2) as bp,
            tc.tile_pool(name="sm", bufs=2) as sp,
            tc.tile_pool(name="ps", bufs=2, space="PSUM") as pp,
        ):
            # ---- constants ----
            qz2 = cp.tile([128, H * WQ], bf, name="qz2")
            nc.sync.dma_start(out=qz2, in_=qz2p[:])
            czd = cp.tile([128, 4 * N], czdt, name="czd")
            nc.sync.dma_start(out=czd, in_=czdp[:])
            sel2 = cp.tile([N, 256], f32, name="sel2")
            nc.sync.dma_start(out=sel2, in_=sel2p[:])
            nsm = cp.tile([128, R], f32, name="nsm")
            nc.sync.dma_start(out=nsm, in_=nsmp[:])

            ones128 = cp.tile([N, 128], f32, name="ones128")
            nc.vector.memset(ones128, 1.0)
            onesb = cp.tile([128, H], f32, name="onesb")
            nc.vector.memset(onesb, 1.0)
            if scan_mode:
                # scan reset mask: 0 at each 57-wide group start, 1 elsewhere
                maskz = cp.tile([128, H * W], bf, name="maskz")
                nc.vector.memset(maskz, 1.0)
                mz0 = bass.AP(tensor=maskz.tensor, offset=maskz.offset,
                              ap=[list(maskz.ap[0]), [W, H]])
                nc.vector.memset(mz0, 0.0)

            # loop-carried state (tail + per-row scalars duplicated on all
            # 128 partitions so the clamp chain never crosses partitions)
            x = cp.tile([128, R], xdt, name="x0")
            nc.vector.memset(x, 1.0 / N)
            if not scalar_tail:
                xt2 = cp.tile([128, N - R], f32, name="xt2")
                nc.vector.memset(xt2, 1.0 / N)
            if not xbf16:
                x2 = cp.tile([128, R], bf, name="x2")
                nc.vector.memset(x2, 1.0 / N)
            tmin2 = cp.tile([128, 1], f32, name="tmin0")
            nc.vector.memset(tmin2, 1.0 / N)
            invn2 = cp.tile([128, 1], f32, name="invn0")
            nc.vector.memset(invn2, 1.0)
            if diag_no_m or diag_no_xna2:
                dconst = cp.tile([128, R], f32, name="dconst")
                nc.vector.memset(dconst, 1.0 / N)

            def body():
                # --- per-row scalar chain ---
                g = sp.tile([128, 1], f32, tag="g", name="g")
                nc.scalar.activation(g, tmin2, ACTF.Copy, bias=0.0, scale=-1.0e6)
                gn = sp.tile([128, 1], f32, tag="gn", name="gn")
                nc.scalar.activation(gn, tmin2, ACTF.Copy, bias=0.0, scale=1.0e6)
                t0 = sp.tile([128, 1], f32, tag="t0", name="t0")
                if pool_t0:
                    nc.gpsimd.tensor_scalar(t0, tmin2, -1.0e6, 0.0,
                                            ALU.mult, ALU.max)
                else:
                    nc.scalar.activation(t0, g, ACTF.Relu, bias=0.0, scale=1.0)

                psc = pp.tile([128, 8], f32, tag="psc", name="psc")
                # psc[:,1] = sum_j g first: it feeds ag -> recip, and recip
                # sits early in the DVE stream
                nc.tensor.matmul(psc[:, 1:2], ones128, g[0:N, 0:1],
                                 start=True, stop=True)
                # psc[:,0] = fused correction (selin-cz-diag_in)@t0
                #            + (selout-diag_out)@g
                nc.tensor.matmul(psc[:, 0:1], sel2[:, 0:128], t0[0:N, 0:1],
                                 start=True, stop=False)
                nc.tensor.matmul(psc[:, 0:1], sel2[:, 128:256], g[0:N, 0:1],
                                 start=False, stop=True)

                ag = sp.tile([128, 1], f32, tag="ag", name="ag")
                nc.scalar.activation(ag, psc[:, 1:2], ACTF.Abs,
                                     bias=0.0, scale=1.0)

                # --- big ops (DVE, 128 partitions, bf16 product) ---
                xsrc = x if xbf16 else x2
                if scan_mode:
                    # product into 57-stride groups; ACT writes G into col 56
                    # of each group; one mask-reset scan computes the grouped
                    # clamped max, landing t1c at col 56 of each group.
                    u = bp.tile([128, H * W], bf, tag="u", name="u")
                    u_main = bass.AP(tensor=u.tensor, offset=u.offset,
                                     ap=[list(u.ap[0]), [W, H], [1, R]])
                    x2b = bass.AP(tensor=xsrc.tensor, offset=xsrc.offset,
                                  ap=[list(xsrc.ap[0]), [0, H], [1, R]])
                    qz2v = bass.AP(tensor=qz2.tensor, offset=qz2.offset,
                                   ap=[list(qz2.ap[0]), [W, H], [1, R]])
                    nc.vector.tensor_tensor(u_main, x2b, qz2v, ALU.mult)
                    nc.vector.reciprocal(invn2, ag)
                    u_gcol = bass.AP(tensor=u.tensor, offset=u.offset + R,
                                     ap=[list(u.ap[0]), [W, H]])
                    nc.scalar.activation(u_gcol, onesb, ACTF.Identity,
                                         bias=g, scale=0.0)
                    us = bp.tile([128, H * W], bf, tag="us", name="us")
                    nc.vector.tensor_tensor_scan(us[:], maskz[:], u[:],
                                                 0.0, ALU.mult, ALU.max)
                    t1ap = bass.AP(tensor=us.tensor, offset=us.offset + R,
                                   ap=[list(us.ap[0]), [W, H]])
                else:
                    US = 64 if aligned else R      # u group stride
                    u = bp.tile([128, H * US], bf, tag="u", name="u")
                    u_main = bass.AP(tensor=u.tensor, offset=u.offset,
                                     ap=[list(u.ap[0]), [US, H], [1, R]])
                    x2b = bass.AP(tensor=xsrc.tensor, offset=xsrc.offset,
                                  ap=[list(xsrc.ap[0]), [0, H], [1, R]])
                    qz2v = bass.AP(tensor=qz2.tensor, offset=qz2.offset,
                                   ap=[list(qz2.ap[0]), [WQ, H], [1, R]])
                    nc.vector.tensor_tensor(u_main, x2b, qz2v, ALU.mult)
                    if recip_after_u:
                        nc.vector.reciprocal(invn2, ag)

                    if aligned:
                        vw = 32
                    else:
                        vw = HB if fold2 else HB + 1
                    v = bp.tile([128, H * vw], bf, tag="v", name="v")
                    if not fold2:
                        v_gcol = bass.AP(tensor=v.tensor, offset=v.offset + HB,
                                         ap=[list(v.ap[0]), [vw, H]])
                        nc.scalar.activation(v_gcol, onesb, ACTF.Identity,
                                             bias=g, scale=0.0)
                    v_main = bass.AP(tensor=v.tensor, offset=v.offset,
                                     ap=[list(v.ap[0]), [vw, H], [1, HB]])
                    u_lo = bass.AP(tensor=u.tensor, offset=u.offset,
                                   ap=[list(u.ap[0]), [US, H], [1, HB]])
                    u_hi = bass.AP(tensor=u.tensor, offset=u.offset + HB,
                                   ap=[list(u.ap[0]), [US, H], [1, HB]])
                    nc.vector.tensor_tensor(v_main, u_lo, u_hi, ALU.max)
                    if fold2:
                        # second 2x fold 28 -> 14 into a 15-stride tile with
                        # the G clamp column, shrinking the 1x reduce input
                        v2 = bp.tile([128, H * 15], bf, tag="v2", name="v2")
                        v2_gcol = bass.AP(tensor=v2.tensor,
                                          offset=v2.offset + 14,
                                          ap=[list(v2.ap[0]), [15, H]])
                        nc.scalar.activation(v2_gcol, onesb, ACTF.Identity,
                                             bias=g, scale=0.0)
                        v2_main = bass.AP(tensor=v2.tensor, offset=v2.offset,
                                          ap=[list(v2.ap[0]), [15, H], [1, 14]])
                        v_lo = bass.AP(tensor=v.tensor, offset=v.offset,
                                       ap=[list(v.ap[0]), [vw, H], [1, 14]])
                        v_hi = bass.AP(tensor=v.tensor, offset=v.offset + 14,
                                       ap=[list(v.ap[0]), [vw, H], [1, 14]])
                        nc.vector.tensor_tensor(v2_main, v_lo, v_hi, ALU.max)
                        red_in = bass.AP(tensor=v2.tensor, offset=v2.offset,
                                         ap=[list(v2.ap[0]), [15, H], [1, 15]])
                    else:
                        red_in = bass.AP(tensor=v.tensor, offset=v.offset,
                                         ap=[list(v.ap[0]), [vw, H],
                                             [1, HB + 1]])
                    t1 = sp.tile([128, H], bf, tag="t1", name="t1")
                    if red_split:
                        # reduce in two halves; the m matmuls for the first
                        # half run on PE while DVE reduces the second half,
                        # hiding ~half the (measured ~1.2us) PE roundtrip
                        HS = H // 2
                        rw = HB + 1
                        t1a = sp.tile([128, HS], bf, tag="t1a", name="t1a")
                        t1b = sp.tile([128, HS], bf, tag="t1b", name="t1b")
                        red_a = bass.AP(tensor=v.tensor, offset=v.offset,
                                        ap=[list(v.ap[0]), [rw, HS], [1, rw]])
                        nc.vector.tensor_reduce(t1a, red_a, AX.X, ALU.max)
                        red_b = bass.AP(tensor=v.tensor,
                                        offset=v.offset + HS * rw,
                                        ap=[list(v.ap[0]), [rw, HS], [1, rw]])
                        m = pp.tile([128, R], f32, tag="m", name="m")
                        nc.tensor.matmul(m[:, 0:HS], czd[0:N, 0:2 * N],
                                         t1a[0:N, :], start=True, stop=True)
                        nc.tensor.matmul(m[:, H:H + HS], czd[:, 2 * N:4 * N],
                                         t1a, start=True, stop=True)
                        nc.vector.tensor_reduce(t1b, red_b, AX.X, ALU.max)
                        nc.tensor.matmul(m[:, HS:H], czd[0:N, 0:2 * N],
                                         t1b[0:N, :], start=True, stop=True)
                        nc.tensor.matmul(m[:, H + HS:R], czd[:, 2 * N:4 * N],
                                         t1b, start=True, stop=True)
                    elif use_pool:
                        nc.vector.pool(t1, red_in, mybir.PoolFunctionType.max)
                    else:
                        nc.vector.tensor_reduce(t1, red_in, AX.X, ALU.max)
                    t1ap = t1
                    # invn for THIS body (un-lagged; any positive scale is
                    # valid by homogeneity).  Sitting between red and x-upd it
                    # fills the PE m-matmul wait instead of delaying v; its
                    # consumers (xna2/xt2/x-upd) all run later than m anyway.
                    if not recip_after_u:
                        nc.vector.reciprocal(invn2, ag)

                if pe_warm:
                    pw = pp.tile([128, 8], f32, tag="pw", name="pw")
                    for _ in range(pe_warm):
                        nc.tensor.matmul(pw[:, 0:8], ones128,
                                         sel2[0:N, 0:8],
                                         start=True, stop=True)
                if red_split:
                    m_done = True
                else:
                    m = pp.tile([128, R], f32, tag="m", name="m")
                if red_split:
                    pass
                elif m_k64 and not scan_mode:
                    # the czd blocks are zero outside one 64-row half, so a
                    # K=64 partition slice halves the Ldweights stream time
                    nc.tensor.matmul(m[:, 0:H], czd[0:N, 0:2 * N],
                                     t1ap[0:N, :], start=True, stop=True)
                    nc.tensor.matmul(m[:, H:R], czd[N:128, 2 * N:4 * N],
                                     t1ap[N:128, :], start=True, stop=True)
                elif m_k64h and not scan_mode:
                    # hybrid: only the partition-0-based half uses K=64
                    nc.tensor.matmul(m[:, 0:H], czd[0:N, 0:2 * N],
                                     t1ap[0:N, :], start=True, stop=True)
                    nc.tensor.matmul(m[:, H:R], czd[:, 2 * N:4 * N], t1ap,
                                     start=True, stop=True)
                else:
                    nc.tensor.matmul(m[:, 0:H], czd[:, 0:2 * N], t1ap,
                                     start=True, stop=True)
                    nc.tensor.matmul(m[:, H:R], czd[:, 2 * N:4 * N], t1ap,
                                     start=True, stop=True)

                # --- assemble the new x with the LAGGED scale (invn from the
                # previous body; valid by homogeneity) ---
                xna = sp.tile([128, R], f32, tag="xna", name="xna")
                nc.gpsimd.tensor_tensor(xna, x, nsm, ALU.mult)
                xna2 = sp.tile([128, R], f32, tag="xna2", name="xna2")
                if pool_psum:
                    nc.gpsimd.tensor_scalar(xna2, xna, psc[:, 0:1], invn2,
                                            ALU.add, ALU.mult)
                else:
                    pscs = sp.tile([128, 1], f32, tag="pscs", name="pscs")
                    nc.scalar.activation(pscs, psc[:, 0:1], ACTF.Copy,
                                         bias=0.0, scale=1.0)
                    nc.gpsimd.tensor_scalar(xna2, xna, pscs, invn2,
                                            ALU.add, ALU.mult)

                # slack-chain ACT ops AFTER pscs so the in-order ACT stream
                # never head-of-line-blocks the x-chain
                e_tail2 = sp.tile([128, 1], f32, tag="e_tail", name="etail")
                nc.scalar.activation(e_tail2, psc[:, 1:2], ACTF.Identity,
                                     bias=gn, scale=1.0)
                # tail: all N-R tail columns of X are identical (equal init,
                # identical affine recurrence), so the tail state IS tmin2
                if scalar_tail:
                    xnt_s = sp.tile([128, 1], f32, tag="xnt", name="xnt")
                    nc.scalar.activation(xnt_s, tmin2, ACTF.Identity,
                                         bias=e_tail2, scale=-1.0e6)
                    nc.gpsimd.tensor_tensor(tmin2, xnt_s, invn2, ALU.mult)
                else:
                    xnt2 = sp.tile([128, N - R], f32, tag="xnt", name="xnt")
                    nc.scalar.activation(xnt2, xt2, ACTF.Identity,
                                         bias=e_tail2, scale=-1.0e6)

                # tail first: its inputs are ready early, so it fills the
                # PE latency of the m matmuls in the in-order DVE stream
                if scalar_tail:
                    pass
                elif act_tail:
                    # tail update on ACT (per-partition AP scale), min-tree
                    # on GPSIMD: the whole tail leaves the DVE stream
                    nc.scalar.activation(xt2, xnt2, ACTF.Copy,
                                         bias=0.0, scale=invn2)
                    tm4 = sp.tile([128, 4], f32, tag="tm4", name="tm4")
                    nc.gpsimd.tensor_tensor(tm4, xt2[:, 0:4], xt2[:, 4:8],
                                            ALU.min)
                    tm2 = sp.tile([128, 2], f32, tag="tm2", name="tm2")
                    nc.gpsimd.tensor_tensor(tm2, tm4[:, 0:2], tm4[:, 2:4],
                                            ALU.min)
                    nc.gpsimd.tensor_tensor(tmin2, tm2[:, 0:1], tm2[:, 1:2],
                                            ALU.min)
                elif pool_xt2:
                    nc.gpsimd.tensor_scalar(xt2, xnt2, invn2,
                                            None, ALU.mult)
                    tm4 = sp.tile([128, 4], f32, tag="tm4", name="tm4")
                    nc.gpsimd.tensor_tensor(tm4, xt2[:, 0:4], xt2[:, 4:8],
                                            ALU.min)
                    tm2 = sp.tile([128, 2], f32, tag="tm2", name="tm2")
                    nc.gpsimd.tensor_tensor(tm2, tm4[:, 0:2], tm4[:, 2:4],
                                            ALU.min)
                    nc.gpsimd.tensor_tensor(tmin2, tm2[:, 0:1], tm2[:, 1:2],
                                            ALU.min)
                else:
                    nc.vector.tensor_scalar(xt2, xnt2, invn2,
                                            None, ALU.mult, ALU.min,
                                            accum_out=tmin2)
                # x = m*invn + xna2 on all 128 partitions (single fused op)
                msrc = dconst if diag_no_m else m
                xna2src = dconst if diag_no_xna2 else xna2
                nc.vector.scalar_tensor_tensor(x, msrc, invn2, xna2src,
                                               ALU.mult, ALU.add)
                if not xbf16:
                    nc.vector.tensor_copy(x2, x)

            with tc.For_i(0, iters // unroll, 1):
                for _ in range(unroll):
                    body()

            # --- final exact normalization (homogeneity: one true L2 norm) ---
            scrf = sp.tile([N, R], f32, tag="scr", name="scrf")
            qrowf = sp.tile([N, 1], f32, tag="qrow", name="qrowf")
            nc.scalar.activation(scrf, x[0:N, :], ACTF.Square, bias=0.0,
                                 scale=1.0, accum_out=qrowf)
            if scalar_tail:
                t64 = tmin2[0:N, 0:1]
                tail_src = bass.AP(tensor=t64.tensor, offset=t64.offset,
                                   ap=[list(t64.ap[0]), [0, N - R]])
            else:
                tail_src = xt2[0:N, :]
            scrft = sp.tile([N, N - R], f32, tag="scrt", name="scrft")
            qrowft = sp.tile([N, 1], f32, tag="qrow2", name="qrowft")
            nc.scalar.activation(scrft, tail_src, ACTF.Square, bias=0.0,
                                 scale=1.0, accum_out=qrowft)
            npf = pp.tile([N, 1], f32, tag="npf", name="npf")
            nc.tensor.matmul(npf, ones128[:, 0:N], qrowf, start=True, stop=False)
            nc.tensor.matmul(npf, ones128[:, 0:N], qrowft, start=False, stop=True)
            snf = sp.tile([N, 1], f32, tag="snf", name="snf")
            nc.scalar.activation(snf, npf, ACTF.Sqrt, bias=0.0, scale=1.0)
            invf = sp.tile([N, 1], f32, tag="invf", name="invf")
            nc.vector.reciprocal(invf, snf)
            xo = sp.tile([N, N], f32, tag="xo", name="xo")
            nc.vector.tensor_scalar(xo[:, 0:R], x[0:N, :], invf, None,
                                    ALU.mult)
            nc.vector.tensor_scalar(xo[:, R:N], tail_src, invf, None,
                                    ALU.mult)
            nc.sync.dma_start(out=xoutp[:], in_=xo)

    nc.finalize()
    return nc


def _get_nc(iters=ITERS):
    key = ("nc", iters, XBF16, POOL_PSUM, POOL_T0, POOL_XT2, M_K64H,
           RECIP_AFTER_U, SCALAR_TAIL)
    if key not in _CACHE:
        _CACHE[key] = _build(iters)
    return _CACHE[key]


def kernel(A_gt, vec_logits, R_int):
    assert int(R_int) == R and A_gt.shape == (N, N)
    ins = _precompute(A_gt, vec_logits)
    nc = _get_nc()

    from concourse.bass_utils import run_bass_kernel_spmd

    core_ids = list(range(8))
    res = run_bass_kernel_spmd(nc, [dict(ins) for _ in core_ids], core_ids)
    out = np.asarray(res.results[0]["xout"], dtype=np.float32).reshape(N, N)
    return out
